# revision 1
# baseline (speedup 1.0000x reference)
"""MLPConv (3x3 valid conv -> 256 -> 256 MLP with ReLU) on 8 TRN2 cores.

Data-parallel over batch: 4 images per core. Per image, the conv is
computed as 9 PSUM-accumulated matmuls (one per filter tap) contracting
over C=128 on the partition dim, with the input transposed on the PE
(identity matmul) into [C, H*W] layout. Both MLP stages keep the
[F, pixels] transposed layout so stage-2 consumes stage-1's output
directly and the per-partition bias lands on the ACT engine's bias port.
Matmuls run as float32r (replicated fp32, 1 cycle/row at N>=256).

Output per core is [F_half, f, img, 62*64 grid]; the host slices the
valid 62 columns and assembles the [F, N, B]-ordered buffer that the
reference reinterprets as [B, 62, 62, F].
"""

import numpy as np

import concourse.bass as bass
import concourse.mybir as mybir
import concourse.tile as tile
from concourse.bass_utils import run_bass_kernel_spmd
from concourse.masks import make_identity

B, H, W, C = 32, 64, 64, 128
F = 256
N_CORES = 8
IMG_PER_CORE = B // N_CORES
HW = H * W                      # 4096 input pixels per image
GRID = 62 * 64                  # 3968 output-grid pixels (64-wide, 62 rows)
NBLK = 8
BLK = GRID // NBLK              # 496 <= 512 fp32 moving-dim limit
XT_PAD = HW + 2 * W + 2         # moving slices reach index 4097

F32 = mybir.dt.float32
F32R = mybir.dt.float32r
BF16 = mybir.dt.bfloat16
RELU = mybir.ActivationFunctionType.Relu


def _split_multi_waits(nc):
    """This container's walrus rejects >1 semaphore wait per instruction
    ("Too many sync wait commands"). Move all but the last wait of each
    instruction onto single-wait NoOps right before it on the same engine."""
    n = 0
    for f in nc.m.functions:
        for bb in f.blocks:
            insts = bb.instructions
            if not any(
                i.sync_info is not None and len(i.sync_info.on_wait) > 1
                for i in insts
            ):
                continue
            new_insts = []
            for inst in insts:
                si = inst.sync_info
                if si is not None and len(si.on_wait) > 1:
                    waits = list(si.on_wait)
                    for k, w in enumerate(waits[:-1]):
                        new_insts.append(
                            mybir.InstNoOp(
                                name=f"{inst.name}-wsplit{k}",
                                engine=inst.engine,
                                bass_nofuse=True,
                                sync_info=mybir.SyncInfo(on_wait=[w], on_update=[]),
                            )
                        )
                        n += 1
                    inst.sync_info = mybir.SyncInfo(
                        on_wait=[waits[-1]], on_update=list(si.on_update)
                    )
                new_insts.append(inst)
            bb.instructions = new_insts
    return n


def build_nc():
    nc = bass.Bass("TRN2", target_bir_lowering=False)
    x = nc.dram_tensor("x", [IMG_PER_CORE, HW, C], F32, kind="ExternalInput").ap()
    w0 = nc.dram_tensor("w0", [9 * C, F], F32, kind="ExternalInput").ap()
    b0 = nc.dram_tensor("b0", [F], F32, kind="ExternalInput").ap()
    w1 = nc.dram_tensor("w1", [F, F], F32, kind="ExternalInput").ap()
    b1 = nc.dram_tensor("b1", [F], F32, kind="ExternalInput").ap()
    out = nc.dram_tensor(
        "out", [2, 128, IMG_PER_CORE, GRID], F32, kind="ExternalOutput"
    ).ap()

    with tile.TileContext(nc) as tc:
        with (
            tc.tile_pool(name="consts", bufs=1) as consts,
            tc.tile_pool(name="xl", bufs=6) as xl,
            tc.tile_pool(name="xlb", bufs=6) as xlbp,
            tc.tile_pool(name="xT", bufs=2) as xT,  # two half-image tiles per img
            tc.tile_pool(name="h1T", bufs=4) as h1T,
            tc.tile_pool(name="outb", bufs=4) as outb,
            tc.tile_pool(name="pt", bufs=2, space="PSUM") as pt,
            tc.tile_pool(name="ps1", bufs=4, space="PSUM") as ps1,
            tc.tile_pool(name="ps2", bufs=2, space="PSUM") as ps2,
        ):
            ident = consts.tile([128, 128], BF16)
            make_identity(nc, ident)

            # first image's input DMAs go first so the PE can start promptly
            xls = {}
            for j in range(4):
                xlt = xl.tile([128, 8, 128], F32, name="xlt")
                nc.sync.dma_start(
                    xlt[:], x[0].rearrange("(b p) c -> p b c", p=128)[:, 8 * j : 8 * (j + 1), :]
                )
                xls[(0, j)] = xlt

            w0f = consts.tile([128, 9, F], F32)
            nc.sync.dma_start(w0f[:], w0.rearrange("(t c) f -> c t f", c=128))
            w0b = consts.tile([128, 9, F], BF16)
            nc.vector.tensor_copy(w0b[:], w0f[:])
            w1f = consts.tile([128, 2, F], F32)
            nc.sync.dma_start(w1f[:], w1.rearrange("(k c) f -> c k f", c=128))
            w1s = consts.tile([128, 2, F], F32R)
            nc.vector.tensor_copy(w1s[:], w1f[:])
            b0s = consts.tile([128, 2], F32)
            nc.sync.dma_start(b0s[:], b0.rearrange("(h f) -> f h", f=128))
            b1s = consts.tile([128, 2], F32)
            nc.sync.dma_start(b1s[:], b1.rearrange("(h f) -> f h", f=128))

            def load_and_transpose(img):
                ximg = x[img].rearrange("(b p) c -> p b c", p=128)
                xTa = xT.tile([128, 17 * 128], BF16, name="xTa")
                xTb = xT.tile([128, 18 * 128], BF16, name="xTb")  # px 1920.. + pad
                xlbs = []
                dmae = [nc.scalar, nc.gpsimd, nc.scalar, nc.gpsimd]
                for j in range(4):
                    if (img, j) in xls:
                        xlt = xls.pop((img, j))
                    else:
                        xlt = xl.tile([128, 8, 128], F32, name="xlt")
                        dmae[j].dma_start(
                            xlt[:], ximg[:, 8 * j : 8 * (j + 1), :]
                        )
                    xlb = xlbp.tile([128, 8, 128], BF16, name="xlb")
                    nc.vector.tensor_copy(xlb[:], xlt[:])
                    xlbs.append(xlb)
                for dst, p0, plist in (
                    (xTa, 0, range(0, 17)),
                    (xTb, 15, range(15, 32)),
                ):
                    for b0i in range(0, 17, 4):
                        batch = list(plist)[b0i : b0i + 4]
                        nb = len(batch)
                        ptt = pt.tile([128, 4, 128], BF16, name="ptt")
                        for q, p in enumerate(batch):
                            nc.tensor.transpose(
                                ptt[:, q, :], xlbs[p // 8][:, p % 8, :], ident[:]
                            )
                        nc.vector.tensor_copy(
                            dst[:, 128 * (batch[0] - p0) : 128 * (batch[0] - p0 + nb)],
                            ptt[:, :nb, :].rearrange("p a b -> p (a b)"),
                        )
                return xTa, xTb

            def stage1(xTa, xTb):
                h1 = []
                for h in range(2):
                    h1t = h1T.tile([128, GRID], F32R, name="h1t")
                    h1.append(h1t)
                for part, xpart, base in ((0, xTa, 0), (1, xTb, 1920)):
                    for h in range(2):
                        for g in (0, 1) if part == 0 else (2, 3):
                            pss = []
                            for bi in range(2):
                                ps1t = ps1.tile([128, BLK], F32, name="ps1t")
                                pss.append(ps1t)
                            for t in range(9):
                                off = (t // 3) * W + (t % 3)
                                wtap = w0b[:, t, 128 * h : 128 * (h + 1)]
                                for bi in range(2):
                                    s = (2 * g + bi) * BLK + off - base
                                    nc.tensor.matmul(
                                        pss[bi][:],
                                        wtap,
                                        xpart[:, s : s + BLK],
                                        start=(t == 0),
                                        stop=(t == 8),
                                    )
                            for bi in range(2):
                                s = (2 * g + bi) * BLK
                                nc.scalar.activation(
                                    h1[h][:, s : s + BLK],
                                    pss[bi][:],
                                    RELU,
                                    bias=b0s[:, h : h + 1],
                                )
                return h1

            def stage2(img, h1):
                for h in range(2):
                    for blk in range(NBLK):
                        s = blk * BLK
                        ps2t = ps2.tile([128, BLK], F32)
                        for k in range(2):
                            nc.tensor.matmul(
                                ps2t[:],
                                w1s[:, k, 128 * h : 128 * (h + 1)],
                                h1[k][:, s : s + BLK],
                                start=(k == 0),
                                stop=(k == 1),
                            )
                        ot = outb.tile([128, BLK], F32)
                        nc.scalar.activation(
                            ot[:], ps2t[:], RELU, bias=b1s[:, h : h + 1]
                        )
                        nc.sync.dma_start(out[h, :, img, s : s + BLK], ot[:])

            # Software pipeline: img i+1's transposes sit between stage1(i)
            # and stage2(i) in the PE stream, hiding the copy/cast latency.
            xab = load_and_transpose(0)
            h1_cur = stage1(*xab)
            for i in range(IMG_PER_CORE):
                if i + 1 < IMG_PER_CORE:
                    xab_next = load_and_transpose(i + 1)
                stage2(i, h1_cur)
                if i + 1 < IMG_PER_CORE:
                    h1_cur = stage1(*xab_next)

    _split_multi_waits(nc)
    return nc


_NC_CACHE = None


def kernel(inputs, w0, b0, w1, b1):
    global _NC_CACHE
    x = np.ascontiguousarray(np.asarray(inputs, dtype=np.float32))
    w0 = np.ascontiguousarray(np.asarray(w0, dtype=np.float32))
    w1 = np.ascontiguousarray(np.asarray(w1, dtype=np.float32))
    b0 = np.ascontiguousarray(np.asarray(b0, dtype=np.float32))
    b1 = np.ascontiguousarray(np.asarray(b1, dtype=np.float32))

    if _NC_CACHE is None:
        _NC_CACHE = build_nc()
    nc = _NC_CACHE

    in_maps = [
        {
            "x": x[c * IMG_PER_CORE : (c + 1) * IMG_PER_CORE].reshape(
                IMG_PER_CORE, HW, C
            ),
            "w0": w0,
            "b0": b0,
            "w1": w1,
            "b1": b1,
        }
        for c in range(N_CORES)
    ]
    res = run_bass_kernel_spmd(nc, in_maps, core_ids=list(range(N_CORES)))

    final = np.empty((B, 62, 62, F), np.float32)
    vf = final.reshape(F, 62 * 62, B)  # the [F, N, B] view the reference reshapes
    for c in range(N_CORES):
        oc = res.results[c]["out"].reshape(F, IMG_PER_CORE, 62, 64)
        oc = oc[:, :, :, :62].reshape(F, IMG_PER_CORE, 62 * 62)
        for i in range(IMG_PER_CORE):
            vf[:, :, c * IMG_PER_CORE + i] = oc[:, i]
    return final



# revision 3
# speedup vs baseline: 1.2122x; 1.2122x over previous
"""MLPConv (3x3 valid conv -> 256 -> 256 MLP with ReLU) on 8 TRN2 cores.

Data-parallel over batch: 4 images per core. The host pre-transposes each
image to [C, H, W] bf16 so the device PE stream is pure matmuls (no
on-device transposes or casts). The conv is 9 PSUM-accumulated matmuls
(one per filter tap) contracting over C=128 on the partition dim; the
moving operand is a 3D access pattern [C, rows, 62] over the [C, 64, 64]
image so only the 62 valid output columns per row are ever computed.
Both stages keep the [F, pixels] transposed layout; stage-1 evacuation
(bias+ReLU) runs on the ACT engine, stage-2 evacuation is split between
ACT and DVE so it keeps pace with the short stage-2 matmul groups.
Output is written bf16 and the host assembles/casts the final buffer.
"""

import numpy as np
import ml_dtypes

import concourse.bass as bass
import concourse.mybir as mybir
import concourse.tile as tile
from concourse.bass_utils import run_bass_kernel_spmd

B, H, W, C = 32, 64, 64, 128
F = 256
N_CORES = 8
IMG = B // N_CORES                  # 4 images per core
OW = 62                             # valid output cols per row
NPX = 62 * 62                       # 3844 valid output pixels per image
ROWS_A = 40                         # image row-split: tile A rows [0, 40)
ROWS_B = 32                         # tile B rows [32, 64)
ROW_B0 = 32

F32 = mybir.dt.float32
BF16 = mybir.dt.bfloat16
RELU = mybir.ActivationFunctionType.Relu
ADD = mybir.AluOpType.add
MAX = mybir.AluOpType.max

# stage blocks: (r0, nrows) covering output rows 0..61
BLOCKS = [(r0, min(8, 62 - r0)) for r0 in range(0, 62, 8)]


def _split_multi_waits(nc):
    """This container's walrus rejects >1 semaphore wait per instruction
    ("Too many sync wait commands"). Move all but the last wait of each
    instruction onto single-wait NoOps right before it on the same engine."""
    n = 0
    for f in nc.m.functions:
        for bb in f.blocks:
            insts = bb.instructions
            if not any(
                i.sync_info is not None and len(i.sync_info.on_wait) > 1
                for i in insts
            ):
                continue
            new_insts = []
            for inst in insts:
                si = inst.sync_info
                if si is not None and len(si.on_wait) > 1:
                    waits = list(si.on_wait)
                    for k, w in enumerate(waits[:-1]):
                        new_insts.append(
                            mybir.InstNoOp(
                                name=f"{inst.name}-wsplit{k}",
                                engine=inst.engine,
                                bass_nofuse=True,
                                sync_info=mybir.SyncInfo(on_wait=[w], on_update=[]),
                            )
                        )
                        n += 1
                    inst.sync_info = mybir.SyncInfo(
                        on_wait=[waits[-1]], on_update=list(si.on_update)
                    )
                new_insts.append(inst)
            bb.instructions = new_insts
    return n


def build_nc():
    nc = bass.Bass("TRN2", target_bir_lowering=False)
    x = nc.dram_tensor("x", [IMG, C, H, W], BF16, kind="ExternalInput").ap()
    w0 = nc.dram_tensor("w0", [C, 9, F], BF16, kind="ExternalInput").ap()
    w1 = nc.dram_tensor("w1", [128, 2, F], BF16, kind="ExternalInput").ap()
    b0 = nc.dram_tensor("b0", [128, 2], F32, kind="ExternalInput").ap()
    b1 = nc.dram_tensor("b1", [128, 2], F32, kind="ExternalInput").ap()
    out = nc.dram_tensor("out", [2, 128, IMG, NPX], BF16, kind="ExternalOutput").ap()

    with tile.TileContext(nc) as tc:
        with (
            tc.tile_pool(name="consts", bufs=1) as consts,
            tc.tile_pool(name="xa", bufs=IMG) as xap,
            tc.tile_pool(name="xb", bufs=IMG) as xbp,
            tc.tile_pool(name="h1", bufs=2) as h1p,
            tc.tile_pool(name="outb", bufs=2) as outp,
            tc.tile_pool(name="ps1", bufs=4, space="PSUM") as ps1,
            tc.tile_pool(name="ps2", bufs=4, space="PSUM") as ps2,
        ):
            # weights/biases on the scalar queue; inputs on sync
            w0t = consts.tile([128, 9, F], BF16)
            nc.scalar.dma_start(w0t[:], w0)
            w1t = consts.tile([128, 2, F], BF16)
            nc.scalar.dma_start(w1t[:], w1)
            b0t = consts.tile([128, 2], F32)
            nc.scalar.dma_start(b0t[:], b0)
            b1t = consts.tile([128, 2], F32)
            nc.scalar.dma_start(b1t[:], b1)

            xts = []
            for i in range(IMG):
                xat = xap.tile([128, ROWS_A, W], BF16, name="xat")
                nc.sync.dma_start(xat[:], x[i, :, 0:ROWS_A, :])
                xbt = xbp.tile([128, ROWS_B, W], BF16, name="xbt")
                nc.sync.dma_start(xbt[:], x[i, :, ROW_B0:H, :])
                xts.append((xat, xbt))

            def stage1(i, h1t):
                xat, xbt = xts[i]
                for r0, nr in BLOCKS:
                    npix = nr * OW
                    p0 = r0 * OW
                    # tile A covers input rows [0, 40); B covers [32, 64)
                    xt, base = (xat, 0) if r0 + nr + 1 < ROWS_A else (xbt, ROW_B0)
                    for h in range(2):
                        ps = ps1.tile([128, 496], F32, name="ps1t")
                        for t in range(9):
                            dy, dx = t // 3, t % 3
                            r = r0 + dy - base
                            nc.tensor.matmul(
                                ps[:, :npix],
                                w0t[:, t, 128 * h : 128 * (h + 1)],
                                xt[:, r : r + nr, dx : dx + OW],
                                start=(t == 0),
                                stop=(t == 8),
                            )
                        nc.scalar.activation(
                            h1t[:, h, p0 : p0 + npix],
                            ps[:, :npix],
                            RELU,
                            bias=b0t[:, h : h + 1],
                        )

            def stage2(i, h1t):
                ot = outp.tile([128, 2, NPX], BF16, name="outt")
                for bi, (r0, nr) in enumerate(BLOCKS):
                    npix = nr * OW
                    p0 = r0 * OW
                    for ho in range(2):
                        ps = ps2.tile([128, 496], F32, name="ps2t")
                        for k in range(2):
                            nc.tensor.matmul(
                                ps[:, :npix],
                                w1t[:, k, 128 * ho : 128 * (ho + 1)],
                                h1t[:, k, p0 : p0 + npix],
                                start=(k == 0),
                                stop=(k == 1),
                            )
                        if bi % 2 == 0:
                            nc.scalar.activation(
                                ot[:, ho, p0 : p0 + npix],
                                ps[:, :npix],
                                RELU,
                                bias=b1t[:, ho : ho + 1],
                            )
                        else:
                            nc.vector.tensor_scalar(
                                ot[:, ho, p0 : p0 + npix],
                                ps[:, :npix],
                                b1t[:, ho : ho + 1],
                                0.0,
                                ADD,
                                MAX,
                            )
                # two half-image stores per (img, half), alternating queues
                half = NPX // 2
                for ho in range(2):
                    nc.gpsimd.dma_start(
                        out[ho, :, i, 0:half], ot[:, ho, 0:half]
                    )
                    nc.sync.dma_start(
                        out[ho, :, i, half:NPX], ot[:, ho, half:NPX]
                    )

            for i in range(IMG):
                h1t = h1p.tile([128, 2, NPX], BF16, name="h1t")
                stage1(i, h1t)
                stage2(i, h1t)

    _split_multi_waits(nc)
    return nc


_NC_CACHE = None


def kernel(inputs, w0, b0, w1, b1):
    global _NC_CACHE
    bf16 = ml_dtypes.bfloat16
    x = np.asarray(inputs, dtype=np.float32)
    # [B, H, W, C] -> [B, C, H, W] bf16, contiguous
    xt = np.ascontiguousarray(x.transpose(0, 3, 1, 2)).astype(bf16)
    w0b = np.ascontiguousarray(
        np.asarray(w0, np.float32).reshape(9, 128, F).transpose(1, 0, 2)
    ).astype(bf16)
    w1b = np.ascontiguousarray(
        np.asarray(w1, np.float32).reshape(2, 128, F).transpose(1, 0, 2)
    ).astype(bf16)
    b0s = np.ascontiguousarray(np.asarray(b0, np.float32).reshape(2, 128).T)
    b1s = np.ascontiguousarray(np.asarray(b1, np.float32).reshape(2, 128).T)

    if _NC_CACHE is None:
        _NC_CACHE = build_nc()
    nc = _NC_CACHE

    in_maps = [
        {
            "x": xt[c * IMG : (c + 1) * IMG],
            "w0": w0b,
            "w1": w1b,
            "b0": b0s,
            "b1": b1s,
        }
        for c in range(N_CORES)
    ]
    res = run_bass_kernel_spmd(nc, in_maps, core_ids=list(range(N_CORES)))

    final = np.empty((B, 62, 62, F), np.float32)
    vf = final.reshape(F, NPX, B)  # the [F, N, B] view the reference reshapes
    for c in range(N_CORES):
        oc = res.results[c]["out"].reshape(F, IMG, NPX)
        for i in range(IMG):
            vf[:, :, c * IMG + i] = oc[:, i]
    return final


# revision 5
# speedup vs baseline: 1.2206x; 1.0069x over previous
"""MLPConv (3x3 valid conv -> 256 -> 256 MLP with ReLU) on 8 TRN2 cores.

Data-parallel over batch: 4 images per core. The host pre-transposes each
image to [C, H, W] bf16 so the device PE stream is pure matmuls (no
on-device transposes or casts). The conv is 9 PSUM-accumulated matmuls
(one per filter tap) contracting over C=128 on the partition dim; the
moving operand is a 3D access pattern [C, rows, 62] over the [C, 64, 64]
image so only the 62 valid output columns per row are ever computed.
Both stages keep the [F, pixels] transposed layout; stage-1 evacuation
(bias+ReLU) runs on the ACT engine, stage-2 evacuation is split between
ACT and DVE so it keeps pace with the short stage-2 matmul groups.
Output is written bf16 and the host assembles/casts the final buffer.
"""

import numpy as np
import ml_dtypes

import concourse.bass as bass
import concourse.mybir as mybir
import concourse.tile as tile
from concourse.bass_utils import run_bass_kernel_spmd

B, H, W, C = 32, 64, 64, 128
F = 256
N_CORES = 8
IMG = B // N_CORES                  # 4 images per core
OW = 62                             # valid output cols per row
NPX = 62 * 62                       # 3844 valid output pixels per image
ROWS_A = 40                         # image row-split: tile A rows [0, 40)
ROWS_B = 32                         # tile B rows [32, 64)
ROW_B0 = 32

F32 = mybir.dt.float32
BF16 = mybir.dt.bfloat16
RELU = mybir.ActivationFunctionType.Relu
ADD = mybir.AluOpType.add
MAX = mybir.AluOpType.max

# stage blocks: (r0, nrows) covering output rows 0..61
BLOCKS = [(r0, min(8, 62 - r0)) for r0 in range(0, 62, 8)]


def _split_multi_waits(nc):
    """This container's walrus rejects >1 semaphore wait per instruction
    ("Too many sync wait commands"). Move all but the last wait of each
    instruction onto single-wait NoOps right before it on the same engine."""
    n = 0
    for f in nc.m.functions:
        for bb in f.blocks:
            insts = bb.instructions
            if not any(
                i.sync_info is not None and len(i.sync_info.on_wait) > 1
                for i in insts
            ):
                continue
            new_insts = []
            for inst in insts:
                si = inst.sync_info
                if si is not None and len(si.on_wait) > 1:
                    waits = list(si.on_wait)
                    for k, w in enumerate(waits[:-1]):
                        new_insts.append(
                            mybir.InstNoOp(
                                name=f"{inst.name}-wsplit{k}",
                                engine=inst.engine,
                                bass_nofuse=True,
                                sync_info=mybir.SyncInfo(on_wait=[w], on_update=[]),
                            )
                        )
                        n += 1
                    inst.sync_info = mybir.SyncInfo(
                        on_wait=[waits[-1]], on_update=list(si.on_update)
                    )
                new_insts.append(inst)
            bb.instructions = new_insts
    return n


def build_nc():
    nc = bass.Bass("TRN2", target_bir_lowering=False)
    x = nc.dram_tensor("x", [IMG, C, H, W], BF16, kind="ExternalInput").ap()
    w0 = nc.dram_tensor("w0", [C, 9, F], BF16, kind="ExternalInput").ap()
    w1 = nc.dram_tensor("w1", [128, 2, F], BF16, kind="ExternalInput").ap()
    b0 = nc.dram_tensor("b0", [128, 2], F32, kind="ExternalInput").ap()
    b1 = nc.dram_tensor("b1", [128, 2], F32, kind="ExternalInput").ap()
    out = nc.dram_tensor("out", [2, 128, IMG, NPX], BF16, kind="ExternalOutput").ap()

    with tile.TileContext(nc) as tc:
        with (
            tc.tile_pool(name="consts", bufs=1) as consts,
            tc.tile_pool(name="xa", bufs=IMG) as xap,
            tc.tile_pool(name="xb", bufs=IMG) as xbp,
            tc.tile_pool(name="h1", bufs=2) as h1p,
            tc.tile_pool(name="outb", bufs=2) as outp,
            tc.tile_pool(name="ps1", bufs=4, space="PSUM") as ps1,
            tc.tile_pool(name="ps2", bufs=4, space="PSUM") as ps2,
        ):
            # sync (earliest HWDGE queue) carries the start-critical chunks in
            # consumption order: first taps + first rows gate the first matmuls
            w0t = consts.tile([128, 9, F], BF16)
            w1t = consts.tile([128, 2, F], BF16)
            b0t = consts.tile([128, 2], F32)
            b1t = consts.tile([128, 2], F32)
            xts = []
            for i in range(IMG):
                xat = xap.tile([128, ROWS_A, W], BF16, name="xat")
                xbt = xbp.tile([128, ROWS_B, W], BF16, name="xbt")
                xts.append((xat, xbt))

            nc.sync.dma_start(w0t[:, 0:3, :], w0[:, 0:3, :])
            nc.sync.dma_start(xts[0][0][:, 0:10, :], x[0, :, 0:10, :])
            nc.sync.dma_start(b0t[:], b0)
            nc.sync.dma_start(w0t[:, 3:9, :], w0[:, 3:9, :])
            nc.sync.dma_start(xts[0][0][:, 10:ROWS_A, :], x[0, :, 10:ROWS_A, :])
            nc.sync.dma_start(xts[0][1][:], x[0, :, ROW_B0:H, :])
            for i in range(1, IMG):
                nc.sync.dma_start(xts[i][0][:], x[i, :, 0:ROWS_A, :])
                nc.sync.dma_start(xts[i][1][:], x[i, :, ROW_B0:H, :])
            nc.scalar.dma_start(w1t[:], w1)
            nc.scalar.dma_start(b1t[:], b1)

            def stage1(i, h1t):
                xat, xbt = xts[i]
                for r0, nr in BLOCKS:
                    npix = nr * OW
                    p0 = r0 * OW
                    # tile A covers input rows [0, 40); B covers [32, 64)
                    xt, base = (xat, 0) if r0 + nr + 1 < ROWS_A else (xbt, ROW_B0)
                    for h in range(2):
                        ps = ps1.tile([128, 496], F32, name="ps1t")
                        for t in range(9):
                            dy, dx = t // 3, t % 3
                            r = r0 + dy - base
                            nc.tensor.matmul(
                                ps[:, :npix],
                                w0t[:, t, 128 * h : 128 * (h + 1)],
                                xt[:, r : r + nr, dx : dx + OW],
                                start=(t == 0),
                                stop=(t == 8),
                            )
                        nc.scalar.activation(
                            h1t[:, h, p0 : p0 + npix],
                            ps[:, :npix],
                            RELU,
                            bias=b0t[:, h : h + 1],
                        )

            def stage2(i, h1t):
                ot = outp.tile([128, 2, NPX], BF16, name="outt")
                # last image streams outputs out per block-pair so the tail
                # after the final matmul is one small store, not a half-image
                qsplits = (
                    [(0, 992), (992, 1984), (1984, 2976), (2976, NPX)]
                    if i == IMG - 1
                    else [(0, 1922), (1922, NPX)]
                )
                qi = 0
                for bi, (r0, nr) in enumerate(BLOCKS):
                    npix = nr * OW
                    p0 = r0 * OW
                    for ho in range(2):
                        ps = ps2.tile([128, 496], F32, name="ps2t")
                        for k in range(2):
                            nc.tensor.matmul(
                                ps[:, :npix],
                                w1t[:, k, 128 * ho : 128 * (ho + 1)],
                                h1t[:, k, p0 : p0 + npix],
                                start=(k == 0),
                                stop=(k == 1),
                            )
                        if bi % 2 == 0:
                            nc.scalar.activation(
                                ot[:, ho, p0 : p0 + npix],
                                ps[:, :npix],
                                RELU,
                                bias=b1t[:, ho : ho + 1],
                            )
                        else:
                            nc.vector.tensor_scalar(
                                ot[:, ho, p0 : p0 + npix],
                                ps[:, :npix],
                                b1t[:, ho : ho + 1],
                                0.0,
                                ADD,
                                MAX,
                            )
                    # flush any output ranges fully evacuated by now
                    done = p0 + npix
                    while qi < len(qsplits) and qsplits[qi][1] <= done:
                        lo, hi = qsplits[qi]
                        for ho in range(2):
                            eng = nc.gpsimd if (qi + ho) % 2 == 0 else nc.sync
                            eng.dma_start(out[ho, :, i, lo:hi], ot[:, ho, lo:hi])
                        qi += 1

            for i in range(IMG):
                h1t = h1p.tile([128, 2, NPX], BF16, name="h1t")
                stage1(i, h1t)
                stage2(i, h1t)

    _split_multi_waits(nc)
    return nc


_NC_CACHE = None


def kernel(inputs, w0, b0, w1, b1):
    global _NC_CACHE
    bf16 = ml_dtypes.bfloat16
    x = np.asarray(inputs, dtype=np.float32)
    # [B, H, W, C] -> [B, C, H, W] bf16, contiguous
    xt = np.ascontiguousarray(x.transpose(0, 3, 1, 2)).astype(bf16)
    w0b = np.ascontiguousarray(
        np.asarray(w0, np.float32).reshape(9, 128, F).transpose(1, 0, 2)
    ).astype(bf16)
    w1b = np.ascontiguousarray(
        np.asarray(w1, np.float32).reshape(2, 128, F).transpose(1, 0, 2)
    ).astype(bf16)
    b0s = np.ascontiguousarray(np.asarray(b0, np.float32).reshape(2, 128).T)
    b1s = np.ascontiguousarray(np.asarray(b1, np.float32).reshape(2, 128).T)

    if _NC_CACHE is None:
        _NC_CACHE = build_nc()
    nc = _NC_CACHE

    in_maps = [
        {
            "x": xt[c * IMG : (c + 1) * IMG],
            "w0": w0b,
            "w1": w1b,
            "b0": b0s,
            "b1": b1s,
        }
        for c in range(N_CORES)
    ]
    res = run_bass_kernel_spmd(nc, in_maps, core_ids=list(range(N_CORES)))

    final = np.empty((B, 62, 62, F), np.float32)
    vf = final.reshape(F, NPX, B)  # the [F, N, B] view the reference reshapes
    for c in range(N_CORES):
        oc = res.results[c]["out"].reshape(F, IMG, NPX)
        for i in range(IMG):
            vf[:, :, c * IMG + i] = oc[:, i]
    return final
